# revision 8
# baseline (speedup 1.0000x reference)
"""Trainium2 Bass kernel for vector-neuron multi-head attention, v7.

Sharding: 8 cores = 4 batches x 2 head-groups (tensor parallel). Each core
projects q/k/z for its 4 heads (128 of 256 output channels) over the full
M=N=2048 tokens, runs attention for those heads, and computes a PARTIAL
final Wo projection (contraction over its 128 channels only). The host
sums the two partials per batch and adds the Wo bias.

The inner loop is ACT(exp)-bound: 128 exps of [128,1024] must stream
back-to-back. Everything else is structured so no other engine ever
stalls them:
  - attention runs in m-512 blocks: each exp spans TWO n-tiles of the
    same m-quarter, so ACT cost is unchanged but the AV accumulator
    shrinks to one PSUM bank, buying pav double-buffering — block
    boundaries are fully decoupled from the eviction chain.
  - PSUM pools split: `pst` (3x[128,2,512], 6 banks) is exclusive to the
    score matmuls — the 3-deep buffering gives each exp a 2-slot input
    lead so HW semaphore propagation never delays the ACT stream; all
    drip PSUM (proj ps, z transposes, y ps) shares one single-buffered
    1-bank `misc` tag. The attention inner loop is
    emitted under tc.high_priority so the Tile scheduler always prefers
    it over dripped work.
  - av eviction: one [97,512] copy to bf16 SBUF (denominator row
    included); recip runs from SBUF (2x DVE mode), the inv row is
    broadcast to 96 partitions by a GPSIMD partition_broadcast, and the
    all-SBUF normalize multiply (4x DVE mode) is deferred into the drip
    queue.
  - two drip queues pumped on alternating slots: the next rep's preamble
    (input loads, projections, fanouts, z transposes) on even slots so
    it spreads over the whole rep (every-slot pumping would overload PE:
    852ns attention + 426ns piece > the ~1045ns ACT slot period), and
    the current rep's normalize + Wo work on odd slots.
  - input loads are chunked per (tensor, d) on the gpsimd DMA queue so
    projection pieces become ready progressively and never flood the
    in-order PE stream; small latency-critical DMAs (fanouts, gathers,
    y stores) keep the sync queue.

Measured: ~170-200us on HW (nrep-slope, +-15% run noise), 132.6us/rep
steady-state in CoreSim (= the 128x1038ns ACT exp roofline, fully
saturated), vs 280822ns baseline.
"""

from collections import deque
from contextlib import ExitStack

import numpy as np

import concourse.bacc as bacc
import concourse.bass as bass
import concourse.tile as tile
from concourse import mybir
from concourse.bass_utils import run_bass_kernel_spmd

FP32 = mybir.dt.float32
BF16 = mybir.dt.float16  # fp16: 10 mantissa bits, same PE speed as bf16
AF = mybir.ActivationFunctionType
ALU = mybir.AluOpType

EMB = 256
HEADS = 8
EPS = 1e-6
B = 4
N = 2048          # tokens (M = N here)
HL = 4            # heads per core
CH = 32           # channels per head
SCALE = 1.0 / np.sqrt(3.0 * CH)
NT = N // 128     # 16 n-tiles
P = 128


def ts(i, s):
    return slice(i * s, (i + 1) * s)


def build_nc(nrep=1):
    nc = bacc.Bacc("TRN2", target_bir_lowering=False, debug=False)

    xs = {
        t: nc.dram_tensor(f"x{t}", [EMB, 3, N], BF16, kind="ExternalInput").ap()
        for t in ("q", "k", "z")
    }
    ws = {
        t: nc.dram_tensor(f"w{t}", [EMB, P], BF16, kind="ExternalInput").ap()
        for t in ("q", "k", "z")
    }
    wo = nc.dram_tensor("wo", [P, EMB], BF16, kind="ExternalInput").ap()
    us = {
        t: nc.dram_tensor(f"u{t}", [P, 3], FP32, kind="ExternalInput").ap()
        for t in ("q", "k", "z")
    }
    ident = nc.dram_tensor("ident", [P, P], BF16, kind="ExternalInput").ap()
    y = nc.dram_tensor("y", [EMB, 3, N], BF16, kind="ExternalOutput").ap()

    xr = {t: x.rearrange("(c p) d t -> p c d t", p=P) for t, x in xs.items()}
    wr = {t: w.rearrange("(c p) e -> p c e", p=P) for t, w in ws.items()}
    yr = y.rearrange("(c p) d t -> p c d t", p=P)

    with tile.TileContext(nc) as tc:
        with ExitStack() as ctx:
            pool = lambda name, bufs, **kw: ctx.enter_context(
                tc.tile_pool(name=name, bufs=bufs, **kw)
            )
            consts = pool("consts", 1)
            xin_pool = pool("xin", 2)
            proj_pool = pool("proj", 2)
            qf_pool = pool("qf", 2)
            kf_pool = pool("kf", 2)
            zft_pool = pool("zft", 2)
            ex_pool = pool("ex", 4)
            av97_pool = pool("av97", 3)
            inv_pool = pool("inv", 1)
            invb_pool = pool("invb", 1)
            outh_pool = pool("outh", 1)
            outall_pool = pool("outall", 1)
            y_pool = pool("ysb", 2)
            pst_pool = pool("pst", 3, space="PSUM")    # 3x2 banks, inner loop only
            pav_pool = pool("pav", 1, space="PSUM")    # 1 bank
            misc_pool = pool("pmisc", 1, space="PSUM") # 1 bank, all drip work

            # constants
            w_sb = {}
            u_sb = {}
            for t in ("q", "k", "z"):
                w_sb[t] = consts.tile([P, 2, P], BF16, tag=f"w{t}", name=f"w{t}_sb")
                nc.sync.dma_start(out=w_sb[t], in_=wr[t])
                u_sb[t] = consts.tile([P, 3], FP32, tag=f"u{t}", name=f"u{t}_sb")
                nc.sync.dma_start(out=u_sb[t], in_=us[t])
            wo_sb = consts.tile([P, EMB], BF16, tag="wo")
            nc.sync.dma_start(out=wo_sb, in_=wo)
            ident_sb = consts.tile([P, P], BF16, tag="ident")
            nc.sync.dma_start(out=ident_sb, in_=ident)

            state = {}  # per-rep tiles, filled by preamble(rep)

            def preamble(rep):
                """Loads + projections + fanouts + z transposes for `rep`,
                yielded as one PE/DMA-sized piece per next()."""
                st_r = {}
                state[rep] = st_r
                xin = {}
                for t in ("q", "k"):
                    xin[t] = xin_pool.tile(
                        [P, 2, 3, N], BF16, tag="xin", name=f"x{t}in"
                    )
                    # bulk loads go on the idle gpsimd queue so the small
                    # latency-critical DMAs (gathers, y stores) on the sync
                    # queue never wait behind them. Chunked per d so the
                    # dripped projection pieces become ready progressively
                    # (one big load would make 12 pieces ready at once and
                    # the scheduler would bunch them, starving the inner
                    # attention loop).
                    for d in range(3):
                        nc.gpsimd.dma_start(
                            out=xin[t][:, :, d, :], in_=xr[t][:, :, d, :]
                        )
                yield
                qf = st_r["qf"] = qf_pool.tile([96, HL, N], BF16, tag="qf", name="qf")
                kf = st_r["kf"] = kf_pool.tile([96, HL, N], BF16, tag="kf", name="kf")
                for t in ("q", "k", "z"):
                    proj = proj_pool.tile([P, 3, N], BF16, tag="proj", name=f"p{t}")
                    for d in range(3):
                        for nq in range(4):
                            ps = misc_pool.tile(
                                [P, 512], FP32, tag="misc", name="projps"
                            )
                            for cc in range(2):
                                nc.tensor.matmul(
                                    ps,
                                    lhsT=w_sb[t][:, cc, :],
                                    rhs=xin[t][:, cc, d, ts(nq, 512)],
                                    start=(cc == 0),
                                    stop=(cc == 1),
                                )
                            nc.vector.tensor_scalar_add(
                                proj[:, d, ts(nq, 512)],
                                ps,
                                u_sb[t][:, d : d + 1],
                            )
                            yield
                        if t in ("q", "k"):
                            for c in range(2):
                                nc.sync.dma_start(
                                    out=st_r[t + "f"][ts(d, 32), :, ts(c, 1024)],
                                    in_=proj[:, d, ts(c, 1024)],
                                )
                    if t == "q":
                        # defer xz so its xin slot (shared with xq) is free
                        xin["z"] = xin_pool.tile(
                            [P, 2, 3, N], BF16, tag="xin", name="xzin"
                        )
                        for d in range(3):
                            nc.gpsimd.dma_start(
                                out=xin["z"][:, :, d, :], in_=xr["z"][:, :, d, :]
                            )
                        yield
                    if t == "z":
                        st_r["pz"] = proj
                zfts = st_r["zfts"] = zft_pool.tile(
                    [P, HL, NT, 98], BF16, tag="zfts", name="zfts"
                )
                nc.vector.memset(
                    zfts.rearrange("p h n c -> p (h n) c")[:, :, 96:97], 1.0
                )
                pz = st_r["pz"]
                for nt in range(NT):
                    # one [128,128] PE transpose per (d, ntile) covers all 4
                    # heads: out cols = (h, ch); the DVE copy then splits
                    # them across the per-head zfts slices
                    zt = misc_pool.tile([P, 3, P], BF16, tag="misc", name="zt")
                    for d in range(3):
                        nc.tensor.transpose(
                            zt[:, d, :], pz[:, d, ts(nt, P)], ident_sb
                        )
                    nc.vector.tensor_copy(
                        zfts[:, :, nt, :96].rearrange("p h (d c) -> p h d c", c=32),
                        zt.rearrange("p d (h c) -> p h d c", c=32),
                    )
                    yield

            def normalize_pieces(rep, mq, h, av97):
                """Deferred per-block normalization: recip on DVE (SBUF 2x
                mode), partition-broadcast on the DMA queue, then one
                all-SBUF bf16 multiply (DVE 4x mode). No PE involvement."""
                st_r = state[rep]
                inv = inv_pool.tile([1, 512], BF16, tag="inv", name="inv")
                with nc.allow_low_precision(reason="softmax inv fp16"):
                    nc.vector.reciprocal(inv, av97[96:97, :])
                yield
                invb = invb_pool.tile([96, 512], BF16, tag="invb", name="invb")
                nc.gpsimd.partition_broadcast(invb, inv)
                yield
                with nc.allow_low_precision(reason="softmax normalize fp16"):
                    nc.vector.tensor_tensor(
                        st_r["outh"][:, h, mq, :],
                        av97[0:96, :],
                        invb,
                        ALU.mult,
                    )
                yield

            def final_proj_pieces(rep, mq):
                """Gather + partial Wo projection for one m-quarter. Must be
                queued after this mq's normalize pieces (the gather reads
                outh, which they write — emission order defines deps)."""
                st_r = state[rep]
                for d in range(3):
                    nc.sync.dma_start(
                        out=st_r["out_all"][:, d, ts(mq, 512)],
                        in_=st_r["outh"][ts(d, 32), :, mq, :],
                    )
                yield
                m0 = 512 * mq
                for d in range(3):
                    for eo in range(2):
                        ps = misc_pool.tile(
                            [P, 512], FP32, tag="misc", name="yps"
                        )
                        nc.tensor.matmul(
                            ps,
                            lhsT=wo_sb[:, ts(eo, P)],
                            rhs=st_r["out_all"][:, d, m0 : m0 + 512],
                            start=True,
                            stop=True,
                        )
                        yp = y_pool.tile([P, 512], BF16, tag="ysb", name="yp")
                        nc.vector.tensor_copy(yp, ps)
                        nc.sync.dma_start(
                            out=yr[:, eo, d, m0 : m0 + 512], in_=yp
                        )
                        yield

            # Two drip queues pumped on alternating slots: the preamble
            # alone would overload PE if dripped every slot (852ns
            # attention + 426ns piece > the 1045ns ACT period), so it gets
            # every other slot and spans the whole rep; normalize/Wo work
            # (naturally spread across the rep) takes the odd slots.
            workP = deque()  # next rep's preamble
            workO = deque()  # current rep's normalize + final Wo
            slot_parity = [0]

            def pump_from(q):
                while q:
                    try:
                        next(q[0])
                        return True
                    except StopIteration:
                        q.popleft()
                return False

            def pump():
                p = slot_parity[0]
                slot_parity[0] ^= 1
                for q in (workP, workO) if p == 0 else (workO, workP):
                    if pump_from(q):
                        return

            def attention(rep):
                # safety: if the drip slots ran out, finish this rep's
                # preamble before consuming its tiles
                while "zfts" not in state.get(rep, {}) and workP:
                    pump_from(workP)
                st_r = state[rep]
                qf, kf, zfts = st_r["qf"], st_r["kf"], st_r["zfts"]
                st_r["out_all"] = outall_pool.tile(
                    [P, 3, N], BF16, tag="outall", name="out_all"
                )
                st_r["outh"] = outh_pool.tile(
                    [96, HL, 4, 512], BF16, tag="outh", name="outh"
                )
                for mq in range(4):
                    for h in range(HL):
                        av = pav_pool.tile([97, 512], FP32, tag="pav", name="av")

                        def av_accum(ntp, ex, av=av, h=h):
                            for j in range(2):
                                nt = 2 * ntp + j
                                nc.tensor.matmul(
                                    av,
                                    lhsT=zfts[:, h, nt, :97],
                                    rhs=ex[:, j, :],
                                    start=(nt == 0),
                                    stop=(nt == NT - 1),
                                )

                        # m-quarter blocks: the exp stays 1024 wide by
                        # spanning TWO n-tiles of the same m-512 block; the
                        # AV accumulator shrinks to one PSUM bank, which
                        # buys pav double-buffering — block boundaries are
                        # fully decoupled from the eviction chain.
                        # AV trails scores by one pair so drip-piece PE
                        # spikes eat the cushion instead of stalling ACT.
                        prev_ex = None
                        for ntp in range(NT // 2):
                            with tc.high_priority(offset=10_000_000):
                                st = pst_pool.tile(
                                    [P, 2, 512], FP32, tag="pst", name="st"
                                )
                                for j in range(2):
                                    nc.tensor.matmul(
                                        st[:, j, :],
                                        lhsT=kf[:, h, ts(2 * ntp + j, P)],
                                        rhs=qf[:, h, ts(mq, 512)],
                                        start=True,
                                        stop=True,
                                    )
                                ex = ex_pool.tile(
                                    [P, 2, 512], BF16, tag="ex", name="ex"
                                )
                                nc.scalar.activation(
                                    ex, st, AF.Exp, scale=float(SCALE)
                                )
                                if prev_ex is not None:
                                    av_accum(ntp - 1, prev_ex)
                                prev_ex = ex
                            pump()
                            pump()
                        with tc.high_priority(offset=10_000_000):
                            av_accum(NT // 2 - 1, prev_ex)
                            # evict av: one copy to bf16 SBUF including the
                            # denominator row 96 (pav is double-buffered,
                            # so this has a whole block of slack)
                            av97 = av97_pool.tile(
                                [97, 512], BF16, tag="av97", name="av97"
                            )
                            nc.vector.tensor_copy(av97, av)
                        workO.append(normalize_pieces(rep, mq, h, av97))
                    workO.append(final_proj_pieces(rep, mq))

            for piece in preamble(0):  # first rep: standalone preamble
                pass
            for rep in range(nrep):
                if rep + 1 < nrep:
                    g = preamble(rep + 1)
                    next(g)  # prime: input-chunk DMAs enqueue at rep start
                    workP.append(g)
                attention(rep)
            while workP or workO:  # drain the last rep's deferred work
                pump()

    nc.compile()
    return nc


_NC_CACHE = {}


def get_nc():
    if "nc" not in _NC_CACHE:
        _NC_CACHE["nc"] = build_nc()
    return _NC_CACHE["nc"]


def _perm_cols(w):
    # [256, 128] -> columns reordered ch-major, head-minor
    return np.ascontiguousarray(
        w.reshape(EMB, HL, CH).transpose(0, 2, 1).reshape(EMB, P)
    )


def _perm_rows(a):
    # [128, ...] -> rows reordered ch-major, head-minor
    s = a.shape
    return np.ascontiguousarray(
        a.reshape(HL, CH, *s[1:]).transpose(1, 0, *range(2, 1 + len(s))).reshape(s)
    )


def make_in_maps(Q, K, Z, Wq_w, Wq_b, Wk_w, Wk_b, Wz_w, Wz_b, Wo_w, Wo_b):
    bf16 = mybir.dt.np(BF16)

    def u_of(b):
        b = np.asarray(b, np.float32)
        return (EPS * b / np.linalg.norm(b, axis=1, keepdims=True)).astype(np.float32)

    uq, uk, uz = u_of(Wq_b), u_of(Wk_b), u_of(Wz_b)
    Qb = np.asarray(Q).astype(bf16)
    Kb = np.asarray(K).astype(bf16)
    Zb = np.asarray(Z).astype(bf16)
    Wqb = np.asarray(Wq_w, np.float32)
    Wkb = np.asarray(Wk_w, np.float32)
    Wzb = np.asarray(Wz_w, np.float32)
    Wob = np.asarray(Wo_w, np.float32)
    ident = np.eye(P, dtype=np.float32).astype(bf16)

    in_maps = []
    for core in range(8):
        b, g = core // 2, core % 2
        cols = slice(P * g, P * (g + 1))
        in_maps.append(
            {
                "xq": np.ascontiguousarray(Qb[b]),
                "xk": np.ascontiguousarray(Kb[b]),
                "xz": np.ascontiguousarray(Zb[b]),
                "wq": _perm_cols(Wqb[:, cols]).astype(bf16),
                "wk": _perm_cols(Wkb[:, cols]).astype(bf16),
                "wz": np.ascontiguousarray(Wzb[:, cols]).astype(bf16),
                "wo": _perm_rows(np.ascontiguousarray(Wob[cols, :])).astype(bf16),
                "uq": _perm_rows(uq[cols]),
                "uk": _perm_rows(uk[cols]),
                "uz": np.ascontiguousarray(uz[cols]),
                "ident": ident,
            }
        )
    return in_maps


def assemble(results, Wo_b=None):
    out = np.empty((B, EMB, 3, N), dtype=np.float32)
    for b in range(B):
        out[b] = results[2 * b]["y"].astype(np.float32) + results[2 * b + 1][
            "y"
        ].astype(np.float32)
    if Wo_b is not None:
        bo = np.asarray(Wo_b, np.float32)
        uo = EPS * bo / np.linalg.norm(bo, axis=1, keepdims=True)
        out += uo[None, :, :, None]
    return out


def kernel(**inputs):
    nc = get_nc()
    in_maps = make_in_maps(**inputs)
    res = run_bass_kernel_spmd(nc, in_maps, list(range(8)))
    return assemble(res.results, Wo_b=inputs["Wo_b"])


if __name__ == "__main__":
    nc = build_nc()
    print("built ok")
